# revision 15
# baseline (speedup 1.0000x reference)
"""Trainium2 Bass kernel for nn_AdaMLP (MoE routing, 64 experts, 2-layer MLP).

Strategy: expert-parallel over 8 NeuronCores; core i owns experts
[8i, 8i+8). The host groups slots by expert (the MoE dispatch), pads
each group to capacity C, and ships per core:
  - the 8 experts' weights quantized to fp8 e3m4 with per-output-channel
    scales (layer-1 scales folded into layer-2 weights, layer-2 scales
    applied on the PSUM->SBUF output op), clip factor per channel chosen
    to minimize weight MSE,
  - transposed slot groups xT in fp16,
  - per-expert output scale/bias columns in f32.
Each core computes, per expert:  H^T = relu(W1q^T-blocks @ xT),
Out^T = s2 * (W2q-blocks @ H^T) + b2, with the fp8 weights as the
stationary matmul operand.  fp8 weights halve the dominant HBM weight
stream (8.4 MB -> 4.2 MB per core) vs bf16; rel err ~1.8e-2 vs the f32
reference (gate 2e-2), deterministic for a fixed input set.

All DMA goes through one HWDGE queue in arrival-need order: xt, scales,
w1[e0], w2[e0], then one merged (w1|w2) DMA per remaining expert, then
per-expert output stores. Activations run only on the Vector engine
(single fused relu per expert; 2 scale ops per expert), so no Scalar
act-table load is needed and the first instruction of the body is the
first DMA issue.
"""

import numpy as np

P = 128                    # SBUF partitions
DIM = 256                  # slot dim
R = 1024                   # hidden dim
E = 64                     # num experts
NCORES = 8
EPC = E // NCORES          # experts per core
DC = DIM // P              # layer-1 contraction chunks (2)
RC = R // P                # r chunks (8)
OC = DIM // P              # output dim chunks (2)
W1C = DC * R               # w1 columns per expert (2048)
W2C = RC * DIM             # w2 columns per expert (2048)
WCOLS = W1C + W2C          # weight columns per expert (4096)

# fp8 e3m4 weight storage roughly halves the (dominant) weight-table DMA
# traffic vs bf16; measured rel err ~1.8e-2 vs the f32 reference (inside
# the 2e-2 gate). Set False for the bf16 fallback (~3.3e-3).
USE_FP8 = True
SHRINK_SEMS = True

_GRAPH_CACHE: dict = {}


def _build_graph(C: int, use_fp8: bool):
    import concourse.bacc as bacc
    import concourse.bass as bass_mod
    import concourse.tile as tile
    from concourse import mybir

    # Shrink the kernel semaphore range: the NEFF epilogue clears every
    # semaphore one EVENT_SEMAPHORE at a time (~90 ns each, split across
    # engines) — with the default 256-sem space that's ~4-5 us of teardown.
    # Use [150, 198) for the kernel and tell walrus codegen the semaphore
    # space ends at 198 so its end-of-NEFF cleanup loop shrinks too.
    if SHRINK_SEMS:
        bass_mod.get_kernel_semaphore_range = lambda: range(150, 198)
        import concourse.bass_utils as bu

        if not getattr(bu.get_walrus_args, "_max_sem_patch", False):
            orig_gwa = bu.get_walrus_args

            def _gwa(*a, **kw):
                return [*orig_gwa(*a, **kw), "--max-sem-num=198"]

            _gwa._max_sem_patch = True
            bu.get_walrus_args = _gwa

    f32 = mybir.dt.float32
    wdt = mybir.dt.float8e3 if use_fp8 else mybir.dt.bfloat16
    xdt = mybir.dt.float16 if use_fp8 else mybir.dt.bfloat16

    mx = mybir.AluOpType.max
    aa = mybir.AluOpType.add
    mm = mybir.AluOpType.mult

    nc = bacc.Bacc(None, target_bir_lowering=False)
    xt_ext = nc.declare_dram_parameter("xt", [P, DC * EPC * C], xdt, isOutput=False)
    wg_ext = nc.declare_dram_parameter("wg", [EPC, P, WCOLS], wdt, isOutput=False)
    # per-expert output scale+bias columns: [s2 | b2] per oc chunk
    sb_ext = nc.declare_dram_parameter("sb", [P, EPC * OC * 2], f32, isOutput=False)
    out_ext = nc.declare_dram_parameter("out", [P, EPC * OC * C], f32, isOutput=True)

    with tile.TileContext(nc) as tc:
        with (
            tc.tile_pool(name="xpool", bufs=1) as xpool,
            tc.tile_pool(name="wpool", bufs=2 * EPC) as wpool,
            tc.tile_pool(name="hpool", bufs=4) as hpool,
            tc.tile_pool(name="opool", bufs=EPC) as opool,
            tc.tile_pool(name="ps1pool", bufs=3, space="PSUM") as ps1pool,
            tc.tile_pool(name="ps2pool", bufs=5, space="PSUM") as ps2pool,
        ):
            # Sync engine's queue carries ONLY the weight stream (in
            # consumption order); xt/scales and the output stores ride the
            # otherwise-idle Scalar engine's queue so they neither delay the
            # weight ring head nor serialize behind it (rings are in-order).
            xt = xpool.tile([P, DC * EPC * C], xdt)
            nc.scalar.dma_start(xt[:], xt_ext[:])
            sb = xpool.tile([P, EPC * OC * 2], f32)
            nc.scalar.dma_start(sb[:], sb_ext[:])
            # One merged (w1|w2) DMA per expert: 4 KB rows, and 8 issues
            # (~0.65 us each on Sync) feed 8 x 1.43 us of data, so the
            # descriptor-generation side never starves the engines (16 split
            # DMAs put the issue pitch at ~the data pitch and cost ~3.5 us).
            wgs = []
            for e in range(EPC):
                wg = wpool.tile([P, WCOLS], wdt)
                nc.sync.dma_start(wg[:], wg_ext[e, :, :])
                wgs.append(wg)
            # single output staging tile, stored with ONE DMA at the end
            # (per-expert stores put 1024 256-byte packets on the engines
            # mid-stream and 7 extra 0.6 us issues on Scalar).
            out_sb = opool.tile([P, EPC * OC * C], f32)

            hs = {}

            def layer2(e):
                # layer 2: Out^T[dim,:] = sum_r W2[r, dim-block] . H^T[r, :]
                w2g = wgs[e][:, W1C:]
                h = hs.pop(e)
                ps2 = ps2pool.tile([P, OC * C], f32)
                for oc_i in range(OC):
                    for rc_i in range(RC):
                        nc.tensor.matmul(
                            ps2[:, oc_i * C : oc_i * C + C],
                            w2g[:, rc_i * DIM + oc_i * P : rc_i * DIM + oc_i * P + P],
                            h[:, rc_i * C : rc_i * C + C],
                            start=(rc_i == 0),
                            stop=(rc_i == RC - 1),
                        )
                # dequant scale on Scalar only (activation Copy with
                # per-partition scale; b2 == 0, checked on host): Vector
                # must stay relu-only — giving it out-ops chains relu(e)
                # behind L2(e-1) through its in-order queue. GPSIMD can't
                # read PSUM.
                base = e * OC * C
                for oc_i in range(OC):
                    nc.scalar.activation(
                        out_sb[:, base + oc_i * C : base + (oc_i + 1) * C],
                        ps2[:, oc_i * C : oc_i * C + C],
                        mybir.ActivationFunctionType.Copy,
                        bias=0.0,
                        scale=sb[:, (e * OC + oc_i) * 2 : (e * OC + oc_i) * 2 + 1],
                    )
                if e == EPC - 2:
                    # store experts 0..6 while expert 7 computes; only
                    # e7's 32 KB store rides the critical tail.
                    nc.scalar.dma_start(
                        out_ext[:, : (EPC - 1) * OC * C],
                        out_sb[:, : (EPC - 1) * OC * C],
                    )

            # Software pipeline: the PE queue is in-order, so L1(e) must be
            # SCHEDULED before L2(e-1) — the PE then runs L1(e) while Vector
            # does relu(e-1) instead of stalling. Emission order alone does
            # not guarantee this (the Tile scheduler re-simulates and its
            # DMA model makes wg(e) look later than it lands), so pace the
            # schedule explicitly: L1(e) floored at the stream cadence,
            # L2(e-1) floored just after L1(e).
            for e in range(EPC):
                w1g = wgs[e][:, :W1C]
                with tc.tile_wait_until(0.0014 * e):
                    # layer 1: H^T[r,:] = sum_d W1[d, r-block] . xT[d, :]
                    # 8 accumulation groups at offsets of one PSUM tile.
                    ps1 = ps1pool.tile([P, RC * C], f32)
                    for rc_i in range(RC):
                        for dc_i in range(DC):
                            nc.tensor.matmul(
                                ps1[:, rc_i * C : rc_i * C + C],
                                w1g[:, dc_i * R + rc_i * P : dc_i * R + rc_i * P + P],
                                xt[:, (dc_i * EPC + e) * C : (dc_i * EPC + e) * C + C],
                                start=(dc_i == 0),
                                stop=(dc_i == DC - 1),
                            )
                    # single fused relu over all 8 chunks (b1 == 0; checked
                    # on host), on Vector; Vector does nothing else.
                    h = hpool.tile([P, RC * C], xdt)
                    nc.vector.tensor_scalar(h[:], ps1[:], 0.0, None, mx)
                    hs[e] = h
                if e >= 1:
                    with tc.tile_wait_until(0.0014 * e + 0.0004):
                        layer2(e - 1)
            with tc.tile_wait_until(0.0014 * EPC):
                layer2(EPC - 1)
            nc.scalar.dma_start(
                out_ext[:, (EPC - 1) * OC * C :], out_sb[:, (EPC - 1) * OC * C :]
            )
    nc.compile()
    return nc


def _get_graph(C: int, use_fp8: bool):
    key = (C, use_fp8)
    if key not in _GRAPH_CACHE:
        _GRAPH_CACHE[key] = _build_graph(C, use_fp8)
    return _GRAPH_CACHE[key]


def _quant_e3m4_chan(w, np_e3m4):
    """Quantize w [n_chan along last axis] to e3m4 with per-channel scale;
    clip factor per channel picked from a small grid to minimize MSE.
    w: (..., K, N) quantized per-column-N over axis -2. Returns (q, s)."""
    amax = np.abs(w).max(axis=-2, keepdims=True)
    amax = np.maximum(amax, 1e-30)
    best_err = None
    best_q = None
    best_s = None
    for g in (1.0, 1.05, 1.1, 1.2, 1.35, 1.5):
        s = amax * (g / 15.5)
        q = np.clip(w / s, -15.5, 15.5).astype(np_e3m4)
        err = ((q.astype(np.float32) * s - w) ** 2).sum(axis=-2, keepdims=True)
        if best_err is None:
            best_err, best_q, best_s = err, q, s
        else:
            m = err < best_err
            best_err = np.where(m, err, best_err)
            best_q = np.where(np.broadcast_to(m, q.shape), q, best_q)
            best_s = np.where(m, s, best_s)
    return best_q, best_s[..., 0, :]


def _run(inputs: dict, trace: bool = False, trace_cores=None, use_bf16=None,
         use_fp8=None, **spmd_kwargs):
    from concourse.bass_utils import run_bass_kernel_spmd
    import ml_dtypes

    if use_fp8 is None:
        use_fp8 = USE_FP8 and not use_bf16

    if use_fp8:
        wdt_np = ml_dtypes.float8_e3m4
        xdt_np = np.float16
    else:
        wdt_np = ml_dtypes.bfloat16
        xdt_np = ml_dtypes.bfloat16

    slots = np.asarray(inputs["slots"], np.float32)
    w1 = np.asarray(inputs["w1"], np.float32)
    b1 = np.asarray(inputs["b1"], np.float32)
    w2 = np.asarray(inputs["w2"], np.float32)
    b2 = np.asarray(inputs["b2"], np.float32)
    indices = np.asarray(inputs["indices"]).astype(np.int64)

    B, K, D = slots.shape
    assert D == DIM and w1.shape == (E, DIM, R) and w2.shape == (E, R, DIM)
    assert not b1.any(), "nonzero b1 needs the per-chunk bias path"
    assert not b2.any(), "nonzero b2 needs the tensor_scalar output path"
    X = slots.reshape(B * K, DIM)
    idx = indices.reshape(B * K)

    counts = np.bincount(idx, minlength=E)
    C = max(int(counts.max()), 16)
    C = ((C + 15) // 16) * 16  # stable capacities -> stable NEFF cache keys

    if use_fp8:
        # per-channel-r scales for w1; fold s1 into w2 rows; per-channel-d
        # scales for w2 applied on-device via the output tensor_scalar.
        w1q, s1 = _quant_e3m4_chan(w1, wdt_np)          # (E,D,R), (E,R)
        w2p = w2 * s1[:, :, None]
        w2q, s2 = _quant_e3m4_chan(w2p, wdt_np)          # (E,R,D), (E,D)
    else:
        w1q = w1.astype(wdt_np)
        w2q = w2.astype(wdt_np)
        s2 = np.ones((E, DIM), np.float32)

    in_maps = []
    pos_lists = []
    for core in range(NCORES):
        xt = np.zeros((P, DC * EPC * C), xdt_np)
        wg = np.empty((EPC, P, WCOLS), wdt_np)
        sb = np.zeros((P, EPC * OC * 2), np.float32)
        core_pos = []
        for e in range(EPC):
            g = core * EPC + e
            pos = np.nonzero(idx == g)[0]
            core_pos.append(pos)
            n = len(pos)
            if n:
                xeT = X[pos].T.astype(xdt_np)  # [DIM, n]
                for dc_i in range(DC):
                    xt[:, (dc_i * EPC + e) * C : (dc_i * EPC + e) * C + n] = (
                        xeT[dc_i * P : (dc_i + 1) * P]
                    )
            wg[e, :, :W1C] = (
                w1q[g].reshape(DC, P, R).transpose(1, 0, 2).reshape(P, W1C)
            )
            wg[e, :, W1C:] = (
                w2q[g].reshape(RC, P, DIM).transpose(1, 0, 2).reshape(P, W2C)
            )
            for oc_i in range(OC):
                k = (e * OC + oc_i) * 2
                sb[:, k] = s2[g, oc_i * P : (oc_i + 1) * P]
                sb[:, k + 1] = b2[g, oc_i * P : (oc_i + 1) * P]
        in_maps.append({"xt": xt, "wg": wg, "sb": sb})
        pos_lists.append(core_pos)

    nc = _get_graph(C, use_fp8)
    res = run_bass_kernel_spmd(
        nc, in_maps, core_ids=list(range(NCORES)), trace=trace,
        trace_cores=trace_cores, **spmd_kwargs,
    )

    out_flat = np.zeros((B * K, DIM), np.float32)
    for core in range(NCORES):
        o = res.results[core]["out"]  # [P, EPC*OC*C]
        for e in range(EPC):
            pos = pos_lists[core][e]
            n = len(pos)
            if n == 0:
                continue
            blk = np.empty((n, DIM), np.float32)
            for oc_i in range(OC):
                cols = o[:, (e * OC + oc_i) * C : (e * OC + oc_i) * C + n]
                blk[:, oc_i * P : (oc_i + 1) * P] = cols.T
            out_flat[pos] = blk
    return out_flat.reshape(B, K, DIM), res


def kernel(**inputs) -> np.ndarray:
    out, _ = _run(inputs)
    return out


# revision 16
# speedup vs baseline: 1.0216x; 1.0216x over previous
"""Trainium2 Bass kernel for nn_AdaMLP (MoE routing, 64 experts, 2-layer MLP).

Strategy: expert-parallel over 8 NeuronCores; core i owns experts
[8i, 8i+8). The host groups slots by expert (the MoE dispatch), pads
each group to capacity C, and ships per core:
  - the 8 experts' weights quantized to fp8 e3m4 with per-output-channel
    scales (layer-1 scales folded into layer-2 weights, layer-2 scales
    applied on the PSUM->SBUF output op), clip factor per channel chosen
    to minimize weight MSE,
  - transposed slot groups xT in fp16,
  - per-expert output scale/bias columns in f32.
Each core computes, per expert:  H^T = relu(W1q^T-blocks @ xT),
Out^T = s2 * (W2q-blocks @ H^T) + b2, with the fp8 weights as the
stationary matmul operand.  fp8 weights halve the dominant HBM weight
stream (8.4 MB -> 4.2 MB per core) vs bf16; rel err ~1.8e-2 vs the f32
reference (gate 2e-2), deterministic for a fixed input set.

All DMA goes through one HWDGE queue in arrival-need order: xt, scales,
w1[e0], w2[e0], then one merged (w1|w2) DMA per remaining expert, then
per-expert output stores. Activations run only on the Vector engine
(single fused relu per expert; 2 scale ops per expert), so no Scalar
act-table load is needed and the first instruction of the body is the
first DMA issue.
"""

import numpy as np

P = 128                    # SBUF partitions
DIM = 256                  # slot dim
R = 1024                   # hidden dim
E = 64                     # num experts
NCORES = 8
EPC = E // NCORES          # experts per core
DC = DIM // P              # layer-1 contraction chunks (2)
RC = R // P                # r chunks (8)
OC = DIM // P              # output dim chunks (2)
W1C = DC * R               # w1 columns per expert (2048)
W2C = RC * DIM             # w2 columns per expert (2048)
WCOLS = W1C + W2C          # weight columns per expert (4096)

# fp8 e3m4 weight storage roughly halves the (dominant) weight-table DMA
# traffic vs bf16; measured rel err ~1.8e-2 vs the f32 reference (inside
# the 2e-2 gate). Set False for the bf16 fallback (~3.3e-3).
USE_FP8 = True
SHRINK_SEMS = True

_GRAPH_CACHE: dict = {}


def _build_graph(C: int, use_fp8: bool):
    import concourse.bacc as bacc
    import concourse.bass as bass_mod
    import concourse.tile as tile
    from concourse import mybir

    # Shrink the kernel semaphore range: the NEFF epilogue clears every
    # semaphore one EVENT_SEMAPHORE at a time (~90 ns each, split across
    # engines) — with the default 256-sem space that's ~4-5 us of teardown.
    # Use [150, 198) for the kernel and tell walrus codegen the semaphore
    # space ends at 198 so its end-of-NEFF cleanup loop shrinks too.
    if SHRINK_SEMS:
        bass_mod.get_kernel_semaphore_range = lambda: range(150, 198)
        import concourse.bass_utils as bu

        if not getattr(bu.get_walrus_args, "_max_sem_patch", False):
            orig_gwa = bu.get_walrus_args

            def _gwa(*a, **kw):
                return [*orig_gwa(*a, **kw), "--max-sem-num=198"]

            _gwa._max_sem_patch = True
            bu.get_walrus_args = _gwa

    f32 = mybir.dt.float32
    wdt = mybir.dt.float8e3 if use_fp8 else mybir.dt.bfloat16
    xdt = mybir.dt.float16 if use_fp8 else mybir.dt.bfloat16

    mx = mybir.AluOpType.max
    aa = mybir.AluOpType.add
    mm = mybir.AluOpType.mult

    nc = bacc.Bacc(None, target_bir_lowering=False)
    xt_ext = nc.declare_dram_parameter("xt", [P, DC * EPC * C], xdt, isOutput=False)
    wg_ext = nc.declare_dram_parameter("wg", [EPC, P, WCOLS], wdt, isOutput=False)
    # per-expert output scale+bias columns: [s2 | b2] per oc chunk
    sb_ext = nc.declare_dram_parameter("sb", [P, EPC * OC * 2], f32, isOutput=False)
    out_ext = nc.declare_dram_parameter("out", [P, EPC * OC * C], f32, isOutput=True)

    with tile.TileContext(nc) as tc:
        with (
            tc.tile_pool(name="xpool", bufs=1) as xpool,
            tc.tile_pool(name="wpool", bufs=2 * EPC) as wpool,
            tc.tile_pool(name="hpool", bufs=4) as hpool,
            tc.tile_pool(name="opool", bufs=EPC) as opool,
            tc.tile_pool(name="ps1pool", bufs=3, space="PSUM") as ps1pool,
            tc.tile_pool(name="ps2pool", bufs=5, space="PSUM") as ps2pool,
        ):
            # Sync engine's queue carries ONLY the weight stream (in
            # consumption order); xt/scales and the output stores ride the
            # otherwise-idle Scalar engine's queue so they neither delay the
            # weight ring head nor serialize behind it (rings are in-order).
            xt = xpool.tile([P, DC * EPC * C], xdt)
            nc.scalar.dma_start(xt[:], xt_ext[:])
            sb = xpool.tile([P, EPC * OC * 2], f32)
            nc.scalar.dma_start(sb[:], sb_ext[:])
            # One merged (w1|w2) DMA per expert: 4 KB rows, and 8 issues
            # (~0.65 us each on Sync) feed 8 x 1.43 us of data, so the
            # descriptor-generation side never starves the engines (16 split
            # DMAs put the issue pitch at ~the data pitch and cost ~3.5 us).
            wgs = []
            for e in range(EPC):
                wg = wpool.tile([P, WCOLS], wdt)
                nc.sync.dma_start(wg[:], wg_ext[e, :, :])
                wgs.append(wg)
            # single output staging tile, stored with ONE DMA at the end
            # (per-expert stores put 1024 256-byte packets on the engines
            # mid-stream and 7 extra 0.6 us issues on Scalar).
            out_sb = opool.tile([P, EPC * OC * C], f32)

            hs = {}

            def layer2(e):
                # layer 2: Out^T[dim,:] = sum_r W2[r, dim-block] . H^T[r, :]
                w2g = wgs[e][:, W1C:]
                h = hs.pop(e)
                ps2 = ps2pool.tile([P, OC * C], f32)
                for oc_i in range(OC):
                    for rc_i in range(RC):
                        nc.tensor.matmul(
                            ps2[:, oc_i * C : oc_i * C + C],
                            w2g[:, rc_i * DIM + oc_i * P : rc_i * DIM + oc_i * P + P],
                            h[:, rc_i * C : rc_i * C + C],
                            start=(rc_i == 0),
                            stop=(rc_i == RC - 1),
                        )
                # dequant scale on Scalar only (activation Copy with
                # per-partition scale; b2 == 0, checked on host): Vector
                # must stay relu-only — giving it out-ops chains relu(e)
                # behind L2(e-1) through its in-order queue. GPSIMD can't
                # read PSUM.
                base = e * OC * C
                for oc_i in range(OC):
                    nc.scalar.activation(
                        out_sb[:, base + oc_i * C : base + (oc_i + 1) * C],
                        ps2[:, oc_i * C : oc_i * C + C],
                        mybir.ActivationFunctionType.Copy,
                        bias=0.0,
                        scale=sb[:, (e * OC + oc_i) * 2 : (e * OC + oc_i) * 2 + 1],
                    )
                if e == EPC - 2:
                    # store experts 0..6 while expert 7 computes; only
                    # e7's 32 KB store rides the critical tail.
                    nc.scalar.dma_start(
                        out_ext[:, : (EPC - 1) * OC * C],
                        out_sb[:, : (EPC - 1) * OC * C],
                    )

            # Software pipeline: the PE queue is in-order, so L1(e) must be
            # SCHEDULED before L2(e-1) — the PE then runs L1(e) while Vector
            # does relu(e-1) instead of stalling. Emission order alone does
            # not guarantee this (the Tile scheduler re-simulates and its
            # DMA model makes wg(e) look later than it lands), so pace the
            # schedule explicitly: L1(e) floored at the stream cadence,
            # L2(e-1) floored just after L1(e).
            for e in range(EPC):
                w1g = wgs[e][:, :W1C]
                with tc.tile_wait_until(0.010 + 0.003 * e):
                    # layer 1: H^T[r,:] = sum_d W1[d, r-block] . xT[d, :]
                    # 8 accumulation groups at offsets of one PSUM tile.
                    ps1 = ps1pool.tile([P, RC * C], f32)
                    for rc_i in range(RC):
                        for dc_i in range(DC):
                            nc.tensor.matmul(
                                ps1[:, rc_i * C : rc_i * C + C],
                                w1g[:, dc_i * R + rc_i * P : dc_i * R + rc_i * P + P],
                                xt[:, (dc_i * EPC + e) * C : (dc_i * EPC + e) * C + C],
                                start=(dc_i == 0),
                                stop=(dc_i == DC - 1),
                            )
                    # single fused relu over all 8 chunks (b1 == 0; checked
                    # on host), on Vector; Vector does nothing else.
                    h = hpool.tile([P, RC * C], xdt)
                    nc.vector.tensor_scalar(h[:], ps1[:], 0.0, None, mx)
                    hs[e] = h
                if e >= 1:
                    with tc.tile_wait_until(0.011 + 0.003 * e):
                        layer2(e - 1)
            with tc.tile_wait_until(0.011 + 0.003 * EPC):
                layer2(EPC - 1)
            nc.scalar.dma_start(
                out_ext[:, (EPC - 1) * OC * C :], out_sb[:, (EPC - 1) * OC * C :]
            )
    nc.compile()
    return nc


def _get_graph(C: int, use_fp8: bool):
    key = (C, use_fp8)
    if key not in _GRAPH_CACHE:
        _GRAPH_CACHE[key] = _build_graph(C, use_fp8)
    return _GRAPH_CACHE[key]


def _quant_e3m4_chan(w, np_e3m4):
    """Quantize w [n_chan along last axis] to e3m4 with per-channel scale;
    clip factor per channel picked from a small grid to minimize MSE.
    w: (..., K, N) quantized per-column-N over axis -2. Returns (q, s)."""
    amax = np.abs(w).max(axis=-2, keepdims=True)
    amax = np.maximum(amax, 1e-30)
    best_err = None
    best_q = None
    best_s = None
    for g in (1.0, 1.05, 1.1, 1.2, 1.35, 1.5):
        s = amax * (g / 15.5)
        q = np.clip(w / s, -15.5, 15.5).astype(np_e3m4)
        err = ((q.astype(np.float32) * s - w) ** 2).sum(axis=-2, keepdims=True)
        if best_err is None:
            best_err, best_q, best_s = err, q, s
        else:
            m = err < best_err
            best_err = np.where(m, err, best_err)
            best_q = np.where(np.broadcast_to(m, q.shape), q, best_q)
            best_s = np.where(m, s, best_s)
    return best_q, best_s[..., 0, :]


def _run(inputs: dict, trace: bool = False, trace_cores=None, use_bf16=None,
         use_fp8=None, **spmd_kwargs):
    from concourse.bass_utils import run_bass_kernel_spmd
    import ml_dtypes

    if use_fp8 is None:
        use_fp8 = USE_FP8 and not use_bf16

    if use_fp8:
        wdt_np = ml_dtypes.float8_e3m4
        xdt_np = np.float16
    else:
        wdt_np = ml_dtypes.bfloat16
        xdt_np = ml_dtypes.bfloat16

    slots = np.asarray(inputs["slots"], np.float32)
    w1 = np.asarray(inputs["w1"], np.float32)
    b1 = np.asarray(inputs["b1"], np.float32)
    w2 = np.asarray(inputs["w2"], np.float32)
    b2 = np.asarray(inputs["b2"], np.float32)
    indices = np.asarray(inputs["indices"]).astype(np.int64)

    B, K, D = slots.shape
    assert D == DIM and w1.shape == (E, DIM, R) and w2.shape == (E, R, DIM)
    assert not b1.any(), "nonzero b1 needs the per-chunk bias path"
    assert not b2.any(), "nonzero b2 needs the tensor_scalar output path"
    X = slots.reshape(B * K, DIM)
    idx = indices.reshape(B * K)

    counts = np.bincount(idx, minlength=E)
    C = max(int(counts.max()), 16)
    C = ((C + 15) // 16) * 16  # stable capacities -> stable NEFF cache keys

    if use_fp8:
        # per-channel-r scales for w1; fold s1 into w2 rows; per-channel-d
        # scales for w2 applied on-device via the output tensor_scalar.
        w1q, s1 = _quant_e3m4_chan(w1, wdt_np)          # (E,D,R), (E,R)
        w2p = w2 * s1[:, :, None]
        w2q, s2 = _quant_e3m4_chan(w2p, wdt_np)          # (E,R,D), (E,D)
    else:
        w1q = w1.astype(wdt_np)
        w2q = w2.astype(wdt_np)
        s2 = np.ones((E, DIM), np.float32)

    in_maps = []
    pos_lists = []
    for core in range(NCORES):
        xt = np.zeros((P, DC * EPC * C), xdt_np)
        wg = np.empty((EPC, P, WCOLS), wdt_np)
        sb = np.zeros((P, EPC * OC * 2), np.float32)
        core_pos = []
        for e in range(EPC):
            g = core * EPC + e
            pos = np.nonzero(idx == g)[0]
            core_pos.append(pos)
            n = len(pos)
            if n:
                xeT = X[pos].T.astype(xdt_np)  # [DIM, n]
                for dc_i in range(DC):
                    xt[:, (dc_i * EPC + e) * C : (dc_i * EPC + e) * C + n] = (
                        xeT[dc_i * P : (dc_i + 1) * P]
                    )
            wg[e, :, :W1C] = (
                w1q[g].reshape(DC, P, R).transpose(1, 0, 2).reshape(P, W1C)
            )
            wg[e, :, W1C:] = (
                w2q[g].reshape(RC, P, DIM).transpose(1, 0, 2).reshape(P, W2C)
            )
            for oc_i in range(OC):
                k = (e * OC + oc_i) * 2
                sb[:, k] = s2[g, oc_i * P : (oc_i + 1) * P]
                sb[:, k + 1] = b2[g, oc_i * P : (oc_i + 1) * P]
        in_maps.append({"xt": xt, "wg": wg, "sb": sb})
        pos_lists.append(core_pos)

    nc = _get_graph(C, use_fp8)
    res = run_bass_kernel_spmd(
        nc, in_maps, core_ids=list(range(NCORES)), trace=trace,
        trace_cores=trace_cores, **spmd_kwargs,
    )

    out_flat = np.zeros((B * K, DIM), np.float32)
    for core in range(NCORES):
        o = res.results[core]["out"]  # [P, EPC*OC*C]
        for e in range(EPC):
            pos = pos_lists[core][e]
            n = len(pos)
            if n == 0:
                continue
            blk = np.empty((n, DIM), np.float32)
            for oc_i in range(OC):
                cols = o[:, (e * OC + oc_i) * C : (e * OC + oc_i) * C + n]
                blk[:, oc_i * P : (oc_i + 1) * P] = cols.T
            out_flat[pos] = blk
    return out_flat.reshape(B, K, DIM), res


def kernel(**inputs) -> np.ndarray:
    out, _ = _run(inputs)
    return out


# revision 17
# speedup vs baseline: 1.0440x; 1.0219x over previous
"""Trainium2 Bass kernel for nn_AdaMLP (MoE routing, 64 experts, 2-layer MLP).

Strategy: expert-parallel over 8 NeuronCores; core i owns experts
[8i, 8i+8). The host groups slots by expert (the MoE dispatch), pads
each group to capacity C, and ships per core:
  - the 8 experts' weights quantized to fp8 e3m4 with per-output-channel
    scales (layer-1 scales folded into layer-2 weights, layer-2 scales
    applied on the PSUM->SBUF output op), clip factor per channel chosen
    to minimize weight MSE,
  - transposed slot groups xT in fp16,
  - per-expert output scale/bias columns in f32.
Each core computes, per expert:  H^T = relu(W1q^T-blocks @ xT),
Out^T = s2 * (W2q-blocks @ H^T) + b2, with the fp8 weights as the
stationary matmul operand.  fp8 weights halve the dominant HBM weight
stream (8.4 MB -> 4.2 MB per core) vs bf16; rel err ~1.8e-2 vs the f32
reference (gate 2e-2), deterministic for a fixed input set.

All DMA goes through one HWDGE queue in arrival-need order: xt, scales,
w1[e0], w2[e0], then one merged (w1|w2) DMA per remaining expert, then
per-expert output stores. Activations run only on the Vector engine
(single fused relu per expert; 2 scale ops per expert), so no Scalar
act-table load is needed and the first instruction of the body is the
first DMA issue.
"""

import numpy as np

P = 128                    # SBUF partitions
DIM = 256                  # slot dim
R = 1024                   # hidden dim
E = 64                     # num experts
NCORES = 8
EPC = E // NCORES          # experts per core
DC = DIM // P              # layer-1 contraction chunks (2)
RC = R // P                # r chunks (8)
OC = DIM // P              # output dim chunks (2)
W1C = DC * R               # w1 columns per expert (2048)
W2C = RC * DIM             # w2 columns per expert (2048)
WCOLS = W1C + W2C          # weight columns per expert (4096)

# fp8 e3m4 weight storage roughly halves the (dominant) weight-table DMA
# traffic vs bf16; measured rel err ~1.8e-2 vs the f32 reference (inside
# the 2e-2 gate). Set False for the bf16 fallback (~3.3e-3).
USE_FP8 = True
SHRINK_SEMS = True

_GRAPH_CACHE: dict = {}


def _build_graph(C: int, use_fp8: bool):
    import concourse.bacc as bacc
    import concourse.bass as bass_mod
    import concourse.tile as tile
    from concourse import mybir

    # Shrink the kernel semaphore range: the NEFF epilogue clears every
    # semaphore one EVENT_SEMAPHORE at a time (~90 ns each, split across
    # engines) — with the default 256-sem space that's ~4-5 us of teardown.
    # Use [150, 198) for the kernel and tell walrus codegen the semaphore
    # space ends at 198 so its end-of-NEFF cleanup loop shrinks too.
    if SHRINK_SEMS:
        bass_mod.get_kernel_semaphore_range = lambda: range(150, 198)
        import concourse.bass_utils as bu

        if not getattr(bu.get_walrus_args, "_max_sem_patch", False):
            orig_gwa = bu.get_walrus_args

            def _gwa(*a, **kw):
                return [*orig_gwa(*a, **kw), "--max-sem-num=198"]

            _gwa._max_sem_patch = True
            bu.get_walrus_args = _gwa

    f32 = mybir.dt.float32
    wdt = mybir.dt.float8e3 if use_fp8 else mybir.dt.bfloat16
    xdt = mybir.dt.float16 if use_fp8 else mybir.dt.bfloat16

    mx = mybir.AluOpType.max
    aa = mybir.AluOpType.add
    mm = mybir.AluOpType.mult

    nc = bacc.Bacc(None, target_bir_lowering=False)
    xt_ext = nc.declare_dram_parameter("xt", [P, DC * EPC * C], xdt, isOutput=False)
    wg_ext = nc.declare_dram_parameter("wg", [EPC, P, WCOLS], wdt, isOutput=False)
    # per-expert output scale+bias columns: [s2 | b2] per oc chunk
    sb_ext = nc.declare_dram_parameter("sb", [P, EPC * OC * 2], f32, isOutput=False)
    out_ext = nc.declare_dram_parameter("out", [P, EPC * OC * C], f32, isOutput=True)

    with tile.TileContext(nc) as tc:
        with (
            tc.tile_pool(name="xpool", bufs=1) as xpool,
            tc.tile_pool(name="wpool", bufs=2 * EPC) as wpool,
            tc.tile_pool(name="hpool", bufs=4) as hpool,
            tc.tile_pool(name="opool", bufs=EPC) as opool,
            tc.tile_pool(name="ps1pool", bufs=3, space="PSUM") as ps1pool,
            tc.tile_pool(name="ps2pool", bufs=5, space="PSUM") as ps2pool,
        ):
            # Sync engine's queue carries ONLY the weight stream (in
            # consumption order); xt/scales and the output stores ride the
            # otherwise-idle Scalar engine's queue so they neither delay the
            # weight ring head nor serialize behind it (rings are in-order).
            xt = xpool.tile([P, DC * EPC * C], xdt)
            nc.scalar.dma_start(xt[:], xt_ext[:])
            sb = xpool.tile([P, EPC * OC * 2], f32)
            nc.scalar.dma_start(sb[:], sb_ext[:])
            # One merged (w1|w2) DMA per expert: 4 KB rows, and 8 issues
            # (~0.65 us each on Sync) feed 8 x 1.43 us of data, so the
            # descriptor-generation side never starves the engines (16 split
            # DMAs put the issue pitch at ~the data pitch and cost ~3.5 us).
            wgs = []
            for e in range(EPC):
                wg = wpool.tile([P, WCOLS], wdt)
                nc.sync.dma_start(wg[:], wg_ext[e, :, :])
                wgs.append(wg)
            # single output staging tile, stored with ONE DMA at the end
            # (per-expert stores put 1024 256-byte packets on the engines
            # mid-stream and 7 extra 0.6 us issues on Scalar).
            out_sb = opool.tile([P, EPC * OC * C], f32)

            hs = {}

            def layer2(e):
                # layer 2: Out^T[dim,:] = sum_r W2[r, dim-block] . H^T[r, :]
                w2g = wgs[e][:, W1C:]
                h = hs.pop(e)
                ps2 = ps2pool.tile([P, OC * C], f32)
                for oc_i in range(OC):
                    for rc_i in range(RC):
                        nc.tensor.matmul(
                            ps2[:, oc_i * C : oc_i * C + C],
                            w2g[:, rc_i * DIM + oc_i * P : rc_i * DIM + oc_i * P + P],
                            h[:, rc_i * C : rc_i * C + C],
                            start=(rc_i == 0),
                            stop=(rc_i == RC - 1),
                        )
                # dequant scale on Scalar only (activation Copy with
                # per-partition scale; b2 == 0, checked on host): Vector
                # must stay relu-only — giving it out-ops chains relu(e)
                # behind L2(e-1) through its in-order queue. GPSIMD can't
                # read PSUM.
                base = e * OC * C
                for oc_i in range(OC):
                    if e == EPC - 1 and oc_i == 1:
                        # last expert only: run oc1 on Vector in parallel
                        # with oc0 on Scalar — no later relu exists for this
                        # to block, and it halves the final dequant latency.
                        nc.vector.tensor_scalar(
                            out_sb[:, base + C : base + 2 * C],
                            ps2[:, C : 2 * C],
                            sb[:, (e * OC + 1) * 2 : (e * OC + 1) * 2 + 1],
                            None,
                            mm,
                        )
                        continue
                    nc.scalar.activation(
                        out_sb[:, base + oc_i * C : base + (oc_i + 1) * C],
                        ps2[:, oc_i * C : oc_i * C + C],
                        mybir.ActivationFunctionType.Copy,
                        bias=0.0,
                        scale=sb[:, (e * OC + oc_i) * 2 : (e * OC + oc_i) * 2 + 1],
                    )
                if e == EPC - 2:
                    # store experts 0..6 while expert 7 computes; only
                    # e7's 32 KB store rides the critical tail.
                    nc.scalar.dma_start(
                        out_ext[:, : (EPC - 1) * OC * C],
                        out_sb[:, : (EPC - 1) * OC * C],
                    )

            # Software pipeline: the PE queue is in-order, so L1(e) must be
            # SCHEDULED before L2(e-1) — the PE then runs L1(e) while Vector
            # does relu(e-1) instead of stalling. Emission order alone does
            # not guarantee this (the Tile scheduler re-simulates and its
            # DMA model makes wg(e) look later than it lands), so pace the
            # schedule explicitly: L1(e) floored at the stream cadence,
            # L2(e-1) floored just after L1(e).
            for e in range(EPC):
                w1g = wgs[e][:, :W1C]
                with tc.tile_wait_until(0.010 + 0.003 * e):
                    # layer 1: H^T[r,:] = sum_d W1[d, r-block] . xT[d, :]
                    # 8 accumulation groups at offsets of one PSUM tile.
                    ps1 = ps1pool.tile([P, RC * C], f32)
                    for rc_i in range(RC):
                        for dc_i in range(DC):
                            nc.tensor.matmul(
                                ps1[:, rc_i * C : rc_i * C + C],
                                w1g[:, dc_i * R + rc_i * P : dc_i * R + rc_i * P + P],
                                xt[:, (dc_i * EPC + e) * C : (dc_i * EPC + e) * C + C],
                                start=(dc_i == 0),
                                stop=(dc_i == DC - 1),
                            )
                    # single fused relu over all 8 chunks (b1 == 0; checked
                    # on host), on Vector; Vector does nothing else.
                    h = hpool.tile([P, RC * C], xdt)
                    nc.vector.tensor_scalar(h[:], ps1[:], 0.0, None, mx)
                    hs[e] = h
                if e >= 1:
                    with tc.tile_wait_until(0.011 + 0.003 * e):
                        layer2(e - 1)
            with tc.tile_wait_until(0.011 + 0.003 * EPC):
                layer2(EPC - 1)
            nc.scalar.dma_start(
                out_ext[:, (EPC - 1) * OC * C :], out_sb[:, (EPC - 1) * OC * C :]
            )
    nc.compile()
    return nc


def _get_graph(C: int, use_fp8: bool):
    key = (C, use_fp8)
    if key not in _GRAPH_CACHE:
        _GRAPH_CACHE[key] = _build_graph(C, use_fp8)
    return _GRAPH_CACHE[key]


def _quant_e3m4_chan(w, np_e3m4):
    """Quantize w [n_chan along last axis] to e3m4 with per-channel scale;
    clip factor per channel picked from a small grid to minimize MSE.
    w: (..., K, N) quantized per-column-N over axis -2. Returns (q, s)."""
    amax = np.abs(w).max(axis=-2, keepdims=True)
    amax = np.maximum(amax, 1e-30)
    best_err = None
    best_q = None
    best_s = None
    for g in (1.0, 1.05, 1.1, 1.2, 1.35, 1.5):
        s = amax * (g / 15.5)
        q = np.clip(w / s, -15.5, 15.5).astype(np_e3m4)
        err = ((q.astype(np.float32) * s - w) ** 2).sum(axis=-2, keepdims=True)
        if best_err is None:
            best_err, best_q, best_s = err, q, s
        else:
            m = err < best_err
            best_err = np.where(m, err, best_err)
            best_q = np.where(np.broadcast_to(m, q.shape), q, best_q)
            best_s = np.where(m, s, best_s)
    return best_q, best_s[..., 0, :]


def _run(inputs: dict, trace: bool = False, trace_cores=None, use_bf16=None,
         use_fp8=None, **spmd_kwargs):
    from concourse.bass_utils import run_bass_kernel_spmd
    import ml_dtypes

    if use_fp8 is None:
        use_fp8 = USE_FP8 and not use_bf16

    if use_fp8:
        wdt_np = ml_dtypes.float8_e3m4
        xdt_np = np.float16
    else:
        wdt_np = ml_dtypes.bfloat16
        xdt_np = ml_dtypes.bfloat16

    slots = np.asarray(inputs["slots"], np.float32)
    w1 = np.asarray(inputs["w1"], np.float32)
    b1 = np.asarray(inputs["b1"], np.float32)
    w2 = np.asarray(inputs["w2"], np.float32)
    b2 = np.asarray(inputs["b2"], np.float32)
    indices = np.asarray(inputs["indices"]).astype(np.int64)

    B, K, D = slots.shape
    assert D == DIM and w1.shape == (E, DIM, R) and w2.shape == (E, R, DIM)
    assert not b1.any(), "nonzero b1 needs the per-chunk bias path"
    assert not b2.any(), "nonzero b2 needs the tensor_scalar output path"
    X = slots.reshape(B * K, DIM)
    idx = indices.reshape(B * K)

    counts = np.bincount(idx, minlength=E)
    C = max(int(counts.max()), 16)
    C = ((C + 15) // 16) * 16  # stable capacities -> stable NEFF cache keys

    if use_fp8:
        # per-channel-r scales for w1; fold s1 into w2 rows; per-channel-d
        # scales for w2 applied on-device via the output tensor_scalar.
        w1q, s1 = _quant_e3m4_chan(w1, wdt_np)          # (E,D,R), (E,R)
        w2p = w2 * s1[:, :, None]
        w2q, s2 = _quant_e3m4_chan(w2p, wdt_np)          # (E,R,D), (E,D)
    else:
        w1q = w1.astype(wdt_np)
        w2q = w2.astype(wdt_np)
        s2 = np.ones((E, DIM), np.float32)

    in_maps = []
    pos_lists = []
    for core in range(NCORES):
        xt = np.zeros((P, DC * EPC * C), xdt_np)
        wg = np.empty((EPC, P, WCOLS), wdt_np)
        sb = np.zeros((P, EPC * OC * 2), np.float32)
        core_pos = []
        for e in range(EPC):
            g = core * EPC + e
            pos = np.nonzero(idx == g)[0]
            core_pos.append(pos)
            n = len(pos)
            if n:
                xeT = X[pos].T.astype(xdt_np)  # [DIM, n]
                for dc_i in range(DC):
                    xt[:, (dc_i * EPC + e) * C : (dc_i * EPC + e) * C + n] = (
                        xeT[dc_i * P : (dc_i + 1) * P]
                    )
            wg[e, :, :W1C] = (
                w1q[g].reshape(DC, P, R).transpose(1, 0, 2).reshape(P, W1C)
            )
            wg[e, :, W1C:] = (
                w2q[g].reshape(RC, P, DIM).transpose(1, 0, 2).reshape(P, W2C)
            )
            for oc_i in range(OC):
                k = (e * OC + oc_i) * 2
                sb[:, k] = s2[g, oc_i * P : (oc_i + 1) * P]
                sb[:, k + 1] = b2[g, oc_i * P : (oc_i + 1) * P]
        in_maps.append({"xt": xt, "wg": wg, "sb": sb})
        pos_lists.append(core_pos)

    nc = _get_graph(C, use_fp8)
    res = run_bass_kernel_spmd(
        nc, in_maps, core_ids=list(range(NCORES)), trace=trace,
        trace_cores=trace_cores, **spmd_kwargs,
    )

    out_flat = np.zeros((B * K, DIM), np.float32)
    for core in range(NCORES):
        o = res.results[core]["out"]  # [P, EPC*OC*C]
        for e in range(EPC):
            pos = pos_lists[core][e]
            n = len(pos)
            if n == 0:
                continue
            blk = np.empty((n, DIM), np.float32)
            for oc_i in range(OC):
                cols = o[:, (e * OC + oc_i) * C : (e * OC + oc_i) * C + n]
                blk[:, oc_i * P : (oc_i + 1) * P] = cols.T
            out_flat[pos] = blk
    return out_flat.reshape(B, K, DIM), res


def kernel(**inputs) -> np.ndarray:
    out, _ = _run(inputs)
    return out
